# revision 52
# baseline (speedup 1.0000x reference)
"""BinaryLlamaDecoderLayer on 8 TRN2 NeuronCores.

Sharding: token-parallel with STRIDED 128-token blocks (core c of a batch
takes global blocks {4j + c%4}), so every core has the same causal profile
and q tile j statically needs only key blocks 0..j (skips 37.5% of the
score/softmax/pv work, perfectly balanced). Weights are baked into the NEFF
as Const tensors (loaded to HBM once at model load, not re-staged per call)
in panel-major layout so each SBUF weight tile fills with ONE DMA of
contiguous-per-partition data (the per-DMA descriptor-generation floor, not
bandwidth, dominates DMA cost). One AllGather (groups of 4) shares rope'd k
(hi/lo bf16) and v across each sequence; the gathered rank-major k/v is
re-ordered to global key order at SBUF-load time. The additive causal mask is
generated on device from a per-row visible-count (iota + compare); softmax P
is transposed with one batched 3D-dst DMA transpose per q tile. Per-call
inputs are just x_t, limits, cos2, srot. Activations feature-major on device;
the q/k path uses a 3-term bf16 hi/lo split for fp32-grade attention scores
(the binarized model's softmax is near-one-hot, so score precision decides
correctness).
"""
import hashlib
import math
import numpy as np
import ml_dtypes

import concourse.bass as bass
import concourse.bacc as bacc
import concourse.mybir as mybir
from concourse import tile

BF = ml_dtypes.bfloat16
F32, BF16 = mybir.dt.float32, mybir.dt.bfloat16
AF = mybir.ActivationFunctionType
OP = mybir.AluOpType

B, S, H = 2, 2048, 2048
NH, NKV, HD = 32, 8, 64
GR = NH // NKV
FF = 5632
EPS = 1e-5
N_CORES = 8
T = (B * S) // N_CORES        # 512 tokens per core
QT = T // 128                 # 4 query tiles per core
KB = S // 512                 # 4 key blocks of 512
SKT = S // 128                # 16 key tiles of 128
HPT = H // 128                # 16 hidden partition tiles
FFT = FF // 128               # 44 ff tiles
ROPE_BASE = 10000.0

_CACHE = {}


def _build_nc(shared, scales, analysis=False, no_collective=False):
    # analysis=True: single-core twin for offline TimelineSim (collective
    # replaced by equivalent local DMA traffic); no_collective=True: 8-core
    # build with the same local-DMA substitution (timing probe only — wrong
    # results). Neither is used for real runs.
    nc = bacc.Bacc("TRN2", target_bir_lowering=False, debug=False,
                   num_devices=(1 if analysis else N_CORES))
    din = {}
    def inp(name, shape, dt):
        din[name] = nc.dram_tensor(name, shape, dt, kind="ExternalInput").ap()
        return din[name]

    x_t   = inp("x_t",   [H, T], F32)          # x^T feature-major
    limits = inp("limits", [128, QT], F32)     # visible-key count per query row
    cos2  = inp("cos2",  [128, T], F32)        # cos stacked x2 (64-row pattern)
    srot  = inp("srot",  [128, T], F32)        # signed sin for rotate-half

    def cw(name):
        return nc.inline_tensor(np.ascontiguousarray(shared[name]), name=name).ap()

    # weights tile-major: row (mt*KT + kt)*128 + p, col c = w^T[kt*128+p, mt*128+c]
    qw_hi = cw("qw_hi")
    qw_lo = cw("qw_lo")
    kw_hi = cw("kw_hi")
    kw_lo = cw("kw_lo")
    vw    = cw("vw")
    ow    = cw("ow")
    gw    = cw("gw")
    uw    = cw("uw")
    dw    = cw("dw")
    out_d = nc.dram_tensor("out", [H, T], F32, kind="ExternalOutput").ap()

    with tile.TileContext(nc) as tc:
        with tc.tile_pool(name="const", bufs=1) as cpool, \
             tc.tile_pool(name="bb", bufs=1) as bpool, \
             tc.tile_pool(name="attn", bufs=1) as apool, \
             tc.tile_pool(name="kv", bufs=2) as kvpool, \
             tc.tile_pool(name="work", bufs=2) as wpool, \
             tc.tile_pool(name="pt", bufs=1) as ptpool, \
             tc.tile_pool(name="wt", bufs=2) as wtpool, \
             tc.tile_pool(name="small", bufs=4) as spool, \
             tc.tile_pool(name="psum", bufs=2, space="PSUM") as pspool, \
             tc.tile_pool(name="dram", bufs=1, space="DRAM") as dpool:

            ones128 = cpool.tile([128, 1], F32, tag="ones128")
            nc.vector.memset(ones128[:], 1.0)
            ones1 = cpool.tile([1, 128], F32, tag="ones1")
            nc.vector.memset(ones1[:], 1.0)
            cos_t = cpool.tile([128, T], F32, tag="cos2")
            nc.sync.dma_start(cos_t[:], cos2[:])
            srot_t = cpool.tile([128, T], F32, tag="srot")
            nc.sync.dma_start(srot_t[:], srot[:])

            eps_t = cpool.tile([1, 1], F32, tag="eps")
            nc.vector.memset(eps_t[:], EPS)

            # ---------- rmsnorm: stats from a DRAM fp32 [H, T] tensor ----------
            def rmsnorm_bcast(src_dram):
                ssum = pspool.tile([1, T], F32, tag="ps")
                for pt in range(HPT):
                    xt = wpool.tile([128, T], F32, tag="xin")
                    nc.sync.dma_start(xt[:], src_dram[pt * 128:(pt + 1) * 128, :])
                    sq = wpool.tile([128, T], F32, tag="hf")
                    nc.vector.tensor_tensor(sq[:], xt[:], xt[:], OP.mult)
                    nc.tensor.matmul(ssum[:], ones128[:], sq[:],
                                     start=(pt == 0), stop=(pt == HPT - 1))
                std = spool.tile([1, T], F32, tag="std", bufs=1)
                nc.scalar.activation(std[:], ssum[:], AF.Sqrt, bias=eps_t[:], scale=1.0 / H)
                rstd = spool.tile([1, T], F32, tag="rstd", bufs=1)
                nc.vector.reciprocal(rstd[:], std[:])
                bc = pspool.tile([128, T], F32, tag="ps")
                nc.tensor.matmul(bc[:], ones1[:], rstd[:], start=True, stop=True)
                bcs = wpool.tile([128, T], F32, tag="bcs", bufs=1)
                nc.vector.tensor_copy(bcs[:], bc[:])
                return bcs

            # ---------- phase 1: rmsnorm1 -> h hi/lo (bb slots 0..31) ----------
            bb = [bpool.tile([128, T], BF16, tag=f"bb{i}", name=f"bb{i}") for i in range(60)]
            h_hi = bb[0:HPT]
            h_lo = bb[HPT:2 * HPT]
            bc1 = rmsnorm_bcast(x_t)
            for pt in range(HPT):
                xt = wpool.tile([128, T], F32, tag="xin")
                nc.sync.dma_start(xt[:], x_t[pt * 128:(pt + 1) * 128, :])
                hf = wpool.tile([128, T], F32, tag="hf")
                nc.vector.tensor_tensor(hf[:], xt[:], bc1[:], OP.mult)
                nc.vector.tensor_copy(h_hi[pt][:], hf[:])
                nc.vector.scalar_tensor_tensor(h_lo[pt][:], hf[:], 1.0, h_hi[pt][:],
                                               OP.mult, OP.subtract)

            # ---------- helper: 3-term projection into psum [128, T] ----------
            def proj3(ps, w_hi_d, w_lo_d, mt):
                n_mm = 3 * HPT
                i = 0
                for cc in range(2):
                    wh = wtpool.tile([128, 128 * 8], BF16, tag="wh", name="wh")
                    wl = wtpool.tile([128, 128 * 8], BF16, tag="wl", name="wl")
                    r0 = (mt * 2 + cc) * 128
                    nc.sync.dma_start(wh[:], w_hi_d[r0:r0 + 128, :])
                    nc.sync.dma_start(wl[:], w_lo_d[r0:r0 + 128, :])
                    for j in range(8):
                        kt = cc * 8 + j
                        for wtile, htile in ((wh, h_hi[kt]), (wh, h_lo[kt]), (wl, h_hi[kt])):
                            nc.tensor.matmul(ps[:], wtile[:, j * 128:(j + 1) * 128],
                                             htile[:], start=(i == 0),
                                             stop=(i == n_mm - 1))
                            i += 1

            # ---------- helper: rope on psum [128, T] (2 heads) ----------
            def rope(ps):
                t1 = wpool.tile([128, T], F32, tag="rope1")
                nc.vector.tensor_tensor(t1[:], ps[:], cos_t[:], OP.mult)
                t2 = wpool.tile([128, T], F32, tag="rope2", bufs=1)
                for g in range(2):
                    o = g * 64
                    nc.vector.tensor_tensor(t2[o:o + 32, :], ps[o + 32:o + 64, :],
                                            srot_t[o:o + 32, :], OP.mult)
                    nc.vector.tensor_tensor(t2[o + 32:o + 64, :], ps[o:o + 32, :],
                                            srot_t[o + 32:o + 64, :], OP.mult)
                nc.vector.tensor_tensor(t1[:], t1[:], t2[:], OP.add)
                return t1

            # ---------- phase 2b: k proj + rope + split (own tokens) ----------
            k_hi_own, k_lo_own = [], []
            for mt in range(NKV * HD // 128):   # 4 tiles
                ps = pspool.tile([128, T], F32, tag="ps")
                proj3(ps, kw_hi, kw_lo, mt)
                kr = rope(ps)
                khi = wpool.tile([128, T], BF16, tag=f"khi{mt}", bufs=1)
                nc.vector.tensor_copy(khi[:], kr[:])
                klo = wpool.tile([128, T], BF16, tag=f"klo{mt}", bufs=1)
                nc.vector.scalar_tensor_tensor(klo[:], kr[:], 1.0, khi[:],
                                               OP.mult, OP.subtract)
                k_hi_own.append(khi)
                k_lo_own.append(klo)

            # ---------- phase 2c: v projection (token-major, bf16) ----------
            v_own = []
            for tmt in range(QT):   # 4 token tiles
                ps = pspool.tile([128, NKV * HD], F32, tag="ps")
                for kt in range(HPT):
                    wv = wtpool.tile([128, NKV * HD], BF16, tag="wv")
                    nc.sync.dma_start(wv[:], vw[kt * 128:(kt + 1) * 128, :])
                    nc.tensor.matmul(ps[:], h_hi[kt][:, tmt * 128:(tmt + 1) * 128],
                                     wv[:], start=(kt == 0), stop=(kt == HPT - 1))
                vt = wpool.tile([128, NKV * HD], BF16, tag=f"vown{tmt}", bufs=1)
                nc.vector.tensor_copy(vt[:], ps[:])
                v_own.append(vt)

            # ---------- phase 3: AllGather k_hi/k_lo/v ----------
            RPR = 1536  # bf16 rows per rank: khi 512, klo 512, v 512
            bounce_in = dpool.tile([RPR, 256], F32, tag="agin")
            bounce_out = dpool.tile([4 * RPR, 256], F32, tag="agout")
            bi_bf = bounce_in.bitcast(BF16)    # [1536, 512] bf16 view
            for mt in range(4):
                nc.sync.dma_start(bi_bf[mt * 128:(mt + 1) * 128, :], k_hi_own[mt][:])
                nc.sync.dma_start(bi_bf[512 + mt * 128:512 + (mt + 1) * 128, :],
                                  k_lo_own[mt][:])
                nc.sync.dma_start(bi_bf[1024 + mt * 128:1024 + (mt + 1) * 128, :],
                                  v_own[mt][:])
            if analysis or no_collective:
                for r in range(4):
                    nc.sync.dma_start(bounce_out[r * RPR:(r + 1) * RPR, :],
                                      bounce_in[:])
            else:
                nc.gpsimd.collective_compute(
                    "AllGather", OP.bypass,
                    replica_groups=[[0, 1, 2, 3], [4, 5, 6, 7]],
                    ins=[bounce_in.opt()],
                    outs=[bounce_out.opt()],
                )
            bo_bf = bounce_out.bitcast(BF16)   # [6144, 512] bf16 view

            # ---------- phase 2a: q proj + rope -> q_stack in DRAM ----------
            # Emitted AFTER the AllGather launch: q-proj depends only on h and
            # the q weights, so its ~0.4 ms of PE work overlaps the collective
            # flight instead of idling at the barrier.
            q_dram = dpool.tile([NH * 128, T], BF16, tag="qstack")
            for mt in range(HPT):        # 2 heads per mt
                ps = pspool.tile([128, T], F32, tag="ps")
                proj3(ps, qw_hi, qw_lo, mt)
                qr = rope(ps)
                qhi = wpool.tile([128, T], BF16, tag="qhi")
                nc.vector.tensor_copy(qhi[:], qr[:])
                qlo = wpool.tile([128, T], BF16, tag="qlo")
                nc.vector.scalar_tensor_tensor(qlo[:], qr[:], 1.0, qhi[:],
                                               OP.mult, OP.subtract)
                for g in range(2):
                    o = g * 64
                    hd_ = 2 * mt + g
                    nc.sync.dma_start(q_dram[hd_ * 128:hd_ * 128 + 64, :],
                                      qhi[o:o + 64, :])
                    nc.sync.dma_start(q_dram[hd_ * 128 + 64:(hd_ + 1) * 128, :],
                                      qlo[o:o + 64, :])

            # additive causal mask built on device: (col >= limit[row]) * -1e9
            limits_t = cpool.tile([128, QT], F32, tag="limits")
            nc.sync.dma_start(limits_t[:], limits[:])
            iota_f = apool.tile([128, S], F32, tag="iota")
            nc.gpsimd.iota(iota_f[:], [[1, S]], channel_multiplier=0,
                           allow_small_or_imprecise_dtypes=True)
            # per-qt visible width only (frees 6KB/partition for kd/kl
            # double-buffering below)
            mask_sb = []
            for qt in range(QT):
                wq = (qt + 1) * 512
                mk = apool.tile([128, wq], BF16, tag=f"mask{qt}")
                nc.vector.tensor_scalar(mk[:], iota_f[:, 0:wq],
                                        limits_t[:, qt:qt + 1], -1e9,
                                        OP.is_ge, OP.mult)
                mask_sb.append(mk)

            attn = []    # 16 tiles [128, T] bf16: attn^T rows = head dims
            for mt in range(HPT):
                attn.append(apool.tile([128, T], BF16, tag=f"attn{mt}", name=f"attn{mt}"))

            # ---------- phase 5: attention ----------
            # Tokens are strided: core (b, lane l) owns global 128-blocks
            # {4j+l}; q tile j therefore sees only keys < (j+1)*512, the same
            # bound on EVERY core, so score/softmax/pv work is statically
            # skipped beyond it. Gathered k/v (rank-major) is re-ordered into
            # global key order at SBUF-load time: global block g lives in rank
            # g%4 at within-rank block g//4.
            for hd_ in range(NH):
                kvh = hd_ // GR
                if hd_ % GR == 0:
                    # stream this kv-head's k into SBUF in global key order;
                    # double-buffered so the next group's loads overlap this
                    # group's score reads
                    kd = kvpool.tile([128, S], BF16, tag="kdup", bufs=2)
                    kl = kvpool.tile([64, S], BF16, tag="klo", bufs=2)
                    for g in range(SKT):
                        r, a = g % 4, g // 4
                        cs = slice(a * 128, (a + 1) * 128)
                        src_hi = bo_bf[r * RPR + kvh * 64: r * RPR + kvh * 64 + 64, cs]
                        src_lo = bo_bf[r * RPR + 512 + kvh * 64:
                                       r * RPR + 512 + kvh * 64 + 64, cs]
                        ds = slice(g * 128, (g + 1) * 128)
                        nc.sync.dma_start(kd[0:64, ds], src_hi)
                        nc.sync.dma_start(kd[64:128, ds], src_hi)
                        nc.sync.dma_start(kl[:, ds], src_lo)
                    v_kv = []
                    for g in range(SKT):
                        r, a = g % 4, g // 4
                        vt = kvpool.tile([128, HD], BF16, tag=f"vk{g}", name=f"vk{g}")
                        nc.sync.dma_start(
                            vt[:],
                            bo_bf[r * RPR + 1024 + a * 128: r * RPR + 1024 + (a + 1) * 128,
                                  kvh * 64:(kvh + 1) * 64])
                        v_kv.append(vt)
                pt_all = ptpool.tile([128, SKT * T], BF16, tag="ptall", name="ptall")
                pt3 = pt_all[:].rearrange("p (a c) -> p a c", a=SKT, c=T)
                qs = wpool.tile([128, T], BF16, tag="qslice")
                nc.sync.dma_start(qs[:], q_dram[hd_ * 128:(hd_ + 1) * 128, :])
                for qt in range(QT):
                    w = (qt + 1) * 512          # visible key width for this q tile
                    nkt = 4 * (qt + 1)
                    qc = slice(qt * 128, (qt + 1) * 128)
                    ps = pspool.tile([128, S], F32, tag="ps")
                    for kb in range(qt + 1):
                        sl = slice(kb * 512, (kb + 1) * 512)
                        nc.tensor.matmul(ps[:, sl], qs[:, qc], kd[:, sl],
                                         start=True, stop=False)
                        nc.tensor.matmul(ps[:, sl], qs[0:64, qc], kl[:, sl],
                                         start=False, stop=True)
                    # in-place mask add on PSUM, then row max, exp, normalize
                    nc.vector.scalar_tensor_tensor(ps[:, 0:w], ps[:, 0:w], 1.0,
                                                   mask_sb[qt][:, 0:w],
                                                   OP.mult, OP.add)
                    mx = spool.tile([128, 1], F32, tag="mx")
                    nc.vector.tensor_reduce(mx[:], ps[:, 0:w],
                                            axis=mybir.AxisListType.X, op=OP.max)
                    nmx = spool.tile([128, 1], F32, tag="nmx")
                    nc.vector.tensor_scalar_mul(nmx[:], mx[:], -1.0)
                    pbf = wpool.tile([128, S], BF16, tag="pbf")
                    sume = spool.tile([128, 1], F32, tag="sume")
                    nc.scalar.activation(pbf[:, 0:w], ps[:, 0:w], AF.Exp, bias=nmx[:],
                                         scale=1.0, accum_out=sume[:])
                    rsum = spool.tile([128, 1], F32, tag="rsum")
                    nc.vector.reciprocal(rsum[:], sume[:])
                    nc.vector.tensor_scalar_mul(pbf[:, 0:w], pbf[:, 0:w], rsum[:])
                    nc.sync.dma_start_transpose(
                        pt3[:, 0:nkt, qt * 128:(qt + 1) * 128],
                        pbf[:, 0:w])
                pav = pspool.tile([64, T], F32, tag="ps")
                for qt in range(QT):
                    nkt = 4 * (qt + 1)
                    qcol = slice(qt * 128, (qt + 1) * 128)
                    for kt in range(nkt):
                        nc.tensor.matmul(
                            pav[:, qcol], v_kv[kt][:],
                            pt_all[:, kt * T + qt * 128:kt * T + (qt + 1) * 128],
                            start=(kt == 0), stop=(kt == nkt - 1))
                o = (hd_ % 2) * 64
                nc.vector.tensor_copy(attn[hd_ // 2][o:o + 64, :], pav[:])

            # ---------- phase 6: o-proj + residual -> hid in DRAM ----------
            hid_d = dpool.tile([H, T], F32, tag="hid")
            for mt in range(HPT):
                ps = pspool.tile([128, T], F32, tag="ps")
                for cc in range(2):
                    wo = wtpool.tile([128, 128 * 8], BF16, tag="wh", name="wo")
                    r0 = (mt * 2 + cc) * 128
                    nc.sync.dma_start(wo[:], ow[r0:r0 + 128, :])
                    for j in range(8):
                        kt = cc * 8 + j
                        nc.tensor.matmul(ps[:], wo[:, j * 128:(j + 1) * 128],
                                         attn[kt][:], start=(kt == 0),
                                         stop=(kt == HPT - 1))
                xt = wpool.tile([128, T], F32, tag="xin")
                nc.sync.dma_start(xt[:], x_t[mt * 128:(mt + 1) * 128, :])
                ht = wpool.tile([128, T], F32, tag="hf")
                nc.vector.tensor_tensor(ht[:], ps[:], xt[:], OP.add)
                nc.sync.dma_start(hid_d[mt * 128:(mt + 1) * 128, :], ht[:])

            # ---------- phase 7: rmsnorm2 -> h2 (bb slots 0..15) ----------
            h2 = bb[0:HPT]
            bc2 = rmsnorm_bcast(hid_d)
            for pt in range(HPT):
                xt = wpool.tile([128, T], F32, tag="xin")
                nc.sync.dma_start(xt[:], hid_d[pt * 128:(pt + 1) * 128, :])
                hf = wpool.tile([128, T], F32, tag="hf")
                nc.vector.tensor_tensor(hf[:], xt[:], bc2[:], OP.mult)
                nc.vector.tensor_copy(h2[pt][:], hf[:])

            # ---------- phase 8: gate/up + silu -> act (bb slots 16..59) ----------
            act = bb[HPT:HPT + FFT]
            for ft in range(FFT):
                psg = pspool.tile([128, T], F32, tag="ps")
                psu = pspool.tile([128, T], F32, tag="ps")
                for cc in range(2):
                    wg = wtpool.tile([128, 128 * 8], BF16, tag="wh", name="wg")
                    wu = wtpool.tile([128, 128 * 8], BF16, tag="wl", name="wu")
                    r0 = (ft * 2 + cc) * 128
                    nc.sync.dma_start(wg[:], gw[r0:r0 + 128, :])
                    nc.sync.dma_start(wu[:], uw[r0:r0 + 128, :])
                    for j in range(8):
                        kt = cc * 8 + j
                        nc.tensor.matmul(psg[:], wg[:, j * 128:(j + 1) * 128],
                                         h2[kt][:], start=(kt == 0), stop=(kt == HPT - 1))
                        nc.tensor.matmul(psu[:], wu[:, j * 128:(j + 1) * 128],
                                         h2[kt][:], start=(kt == 0), stop=(kt == HPT - 1))
                gs = wpool.tile([128, T], BF16, tag="gs")
                nc.scalar.activation(gs[:], psg[:], AF.Silu)
                nc.vector.tensor_tensor(act[ft][:], gs[:], psu[:], OP.mult)

            # ---------- phase 9: down + residual -> out ----------
            for mt in range(HPT):
                ps = pspool.tile([128, T], F32, tag="ps")
                for kc in range(4):          # 11 kt per chunk
                    wd = wtpool.tile([128, 128 * 11], BF16, tag="wd")
                    r0 = (mt * 4 + kc) * 128
                    nc.sync.dma_start(wd[:], dw[r0:r0 + 128, :])
                    for j in range(11):
                        kt = kc * 11 + j
                        nc.tensor.matmul(ps[:], wd[:, j * 128:(j + 1) * 128],
                                         act[kt][:], start=(kt == 0),
                                         stop=(kt == FFT - 1))
                xt = wpool.tile([128, T], F32, tag="xin")
                nc.sync.dma_start(xt[:], hid_d[mt * 128:(mt + 1) * 128, :])
                ot = wpool.tile([128, T], F32, tag="hf")
                nc.vector.tensor_tensor(ot[:], ps[:], xt[:], OP.add)
                nc.sync.dma_start(out_d[mt * 128:(mt + 1) * 128, :], ot[:])

    nc.compile()
    return nc


def _shared_weights(inputs):
    kk = np.float32(inputs["kk"])
    aa = np.float32(inputs["aa"])
    def binw(w):
        return (aa * np.clip(kk * np.asarray(w, dtype=np.float32), -1.0, 1.0))
    ln1 = np.asarray(inputs["ln1_w"], dtype=np.float32)
    ln2 = np.asarray(inputs["ln2_w"], dtype=np.float32)
    qw = binw(inputs["q_w"]) * ln1[None, :] / np.float32(math.sqrt(HD))
    kw = binw(inputs["k_w"]) * ln1[None, :]
    vw = binw(inputs["v_w"]) * ln1[None, :]
    ow = binw(inputs["o_w"])
    gw = binw(inputs["gate_w"]) * ln2[None, :]
    uw = binw(inputs["up_w"]) * ln2[None, :]
    dw = binw(inputs["down_w"])

    def split(w):
        hi = w.astype(BF)
        lo = (w - hi.astype(np.float32)).astype(BF)
        return np.ascontiguousarray(hi), np.ascontiguousarray(lo)

    def panels(wt, kpp):
        # wt: [K, M] -> panel-major [n_mt*n_pan*128, kpp*128] where panel
        # (mt, pan) row p, col j*128+c = wt[(pan*kpp+j)*128+p, mt*128+c].
        # Each SBUF weight tile then fills with ONE dma whose per-partition
        # data is contiguous (kpp*128 elems), minimizing DMA descriptors.
        K, M = wt.shape
        n_kt, n_mt = K // 128, M // 128
        n_pan = n_kt // kpp
        w5 = wt.reshape(n_pan, kpp, 128, n_mt, 128).transpose(3, 0, 2, 1, 4)
        return np.ascontiguousarray(w5.reshape(n_mt * n_pan * 128, kpp * 128))

    qw_hi, qw_lo = split(qw.T)     # [H, H]
    kw_hi, kw_lo = split(kw.T)     # [H, 512]
    shared = {
        "qw_hi": panels(qw_hi, 8), "qw_lo": panels(qw_lo, 8),
        "kw_hi": panels(kw_hi, 8), "kw_lo": panels(kw_lo, 8),
        "vw": np.ascontiguousarray(vw.T.astype(BF)),
        "ow": panels(ow.T.astype(BF), 8),
        "gw": panels(gw.T.astype(BF), 8),
        "uw": panels(uw.T.astype(BF), 8),
        "dw": panels(dw.T.astype(BF), 11),
    }
    return shared, {}


def _token_idx(c):
    """Global token indices (within batch b=c//4) owned by core c, in on-core
    order: strided 128-blocks {4j + (c%4)} so every core has the same causal
    profile per q tile."""
    l = c % 4
    return np.concatenate([np.arange((4 * j + l) * 128, (4 * j + l + 1) * 128)
                           for j in range(QT)])


def _activation_maps(inputs):
    x = np.asarray(inputs["hidden_states"], dtype=np.float32)
    mask = np.asarray(inputs["attention_mask"], dtype=np.float32)
    pos = np.asarray(inputs["position_ids"], dtype=np.int32)

    in_maps = []
    for c in range(N_CORES):
        b = c // 4
        sl = _token_idx(c)
        inv = (1.0 / (ROPE_BASE ** (np.arange(0, HD, 2, dtype=np.float32) / np.float32(HD))))
        fr = pos[b, sl].astype(np.float32)[:, None] * inv[None, :]   # [T, 32]
        emb = np.concatenate([fr, fr], axis=-1)                      # [T, 64]
        cos = np.cos(emb).astype(np.float32).T                       # [64, T]
        sin = np.sin(emb).astype(np.float32).T                       # [64, T]
        srot = np.concatenate([-sin[0:32], sin[32:64]], axis=0)      # [64, T]
        # visible-key count per query row, from the actual input mask
        vis = (mask[b, 0, sl, :] > -0.5).sum(axis=-1).astype(np.float32)  # [T]
        limits = np.ascontiguousarray(vis.reshape(QT, 128).T)             # [128, QT]
        in_maps.append({
            "x_t": np.ascontiguousarray(x[b, sl].T),
            "limits": limits,
            "cos2": np.ascontiguousarray(np.concatenate([cos, cos], axis=0)),
            "srot": np.ascontiguousarray(np.concatenate([srot, srot], axis=0)),
        })
    return in_maps


def _weights_key(shared, scales):
    h = hashlib.blake2b(digest_size=16)
    for k in sorted(shared):
        h.update(k.encode())
        h.update(np.ascontiguousarray(shared[k]).tobytes())
    for k in sorted(scales):
        h.update(k.encode())
        h.update(np.float32(scales[k]).tobytes())
    return h.hexdigest()


def prepare(inputs):
    """Build (or reuse) the compiled nc + jitted runner for these weights.

    Returns (nc, in_maps) where in_maps carry only the per-core activations.
    """
    shared, scales = _shared_weights(inputs)
    key = _weights_key(shared, scales)
    if _CACHE.get("key") != key:
        _CACHE.clear()
        _CACHE["key"] = key
        _CACHE["nc"] = _build_nc(shared, scales)
    return _CACHE["nc"], _activation_maps(inputs)


def _get_runner(nc):
    """Jitted SPMD runner (cached; safe for repeat kernel() calls even though
    the neuron lowering mutates Const allocations in place)."""
    if "runner" in _CACHE:
        return _CACHE["runner"]
    import jax
    from jax.sharding import Mesh, PartitionSpec
    from jax.experimental.shard_map import shard_map
    from concourse.bass2jax import (_bass_exec_p, install_neuronx_cc_hook,
                                    partition_id_tensor)
    install_neuronx_cc_hook()

    partition_name = nc.partition_id_tensor.name if nc.partition_id_tensor else None
    in_names, out_names, out_avals = [], [], []
    for alloc in nc.m.functions[0].allocations:
        if not isinstance(alloc, mybir.MemoryLocationSet):
            continue
        name = alloc.memorylocations[0].name
        if alloc.kind == "ExternalInput":
            if name != partition_name:
                in_names.append(name)
        elif alloc.kind == "ExternalOutput":
            out_names.append(name)
            shape = tuple(alloc.tensor_shape)
            dtype = mybir.dt.np(alloc.dtype)
            out_avals.append(jax.core.ShapedArray(shape, dtype))
    # outputs get fresh NKI hbm buffers in the neuron lowering; no zero-init
    # operands needed (the kernel writes every element of out)
    all_in_names = list(in_names)
    if partition_name is not None:
        all_in_names.append(partition_name)

    def _body(*args):
        operands = list(args)
        if partition_name is not None:
            operands.append(partition_id_tensor())
        outs = _bass_exec_p.bind(
            *operands,
            out_avals=tuple(out_avals),
            in_names=tuple(all_in_names),
            out_names=tuple(out_names),
            lowering_input_output_aliases=(),
            sim_require_finite=True,
            sim_require_nnan=True,
            nc=nc,
        )
        return tuple(outs)

    devices = jax.devices()[:N_CORES]
    mesh = Mesh(np.asarray(devices), ("core",))
    fn = jax.jit(shard_map(_body, mesh=mesh,
                           in_specs=(PartitionSpec("core"),) * len(in_names),
                           out_specs=(PartitionSpec("core"),) * len(out_names),
                           check_rep=False), keep_unused=True)
    runner = {"fn": fn, "in_names": in_names, "out_names": out_names,
              "jax": jax}
    _CACHE["runner"] = runner
    return runner


def kernel(**inputs):
    nc, in_maps = prepare(inputs)
    r = _get_runner(nc)
    jax = r["jax"]
    concat_in = [np.concatenate([np.asarray(in_maps[c][k]) for c in range(N_CORES)],
                                axis=0) for k in r["in_names"]]
    outs = jax.block_until_ready(r["fn"](*concat_in))
    oidx = r["out_names"].index("out")
    res = np.asarray(outs[oidx]).reshape(N_CORES, H, T)
    out = np.empty((B, S, H), dtype=np.float32)
    for c in range(N_CORES):
        out[c // 4, _token_idx(c), :] = res[c].T
    return out


# revision 53
# speedup vs baseline: 1.0220x; 1.0220x over previous
"""BinaryLlamaDecoderLayer on 8 TRN2 NeuronCores.

Sharding: token-parallel with STRIDED 128-token blocks (core c of a batch
takes global blocks {4j + c%4}), so every core has the same causal profile
and q tile j statically needs only key blocks 0..j (skips 37.5% of the
score/softmax/pv work, perfectly balanced). Weights are baked into the NEFF
as Const tensors (loaded to HBM once at model load, not re-staged per call)
in panel-major layout so each SBUF weight tile fills with ONE DMA of
contiguous-per-partition data (the per-DMA descriptor-generation floor, not
bandwidth, dominates DMA cost). One AllGather (groups of 4) shares rope'd k
(hi/lo bf16) and v across each sequence; the gathered rank-major k/v is
re-ordered to global key order at SBUF-load time. The additive causal mask is
generated on device from a per-row visible-count (iota + compare); softmax P
is transposed with one batched 3D-dst DMA transpose per q tile. Per-call
inputs are just x_t, limits, cos2, srot. Activations feature-major on device;
the q/k path uses a 3-term bf16 hi/lo split for fp32-grade attention scores
(the binarized model's softmax is near-one-hot, so score precision decides
correctness).
"""
import hashlib
import math
import numpy as np
import ml_dtypes

import concourse.bass as bass
import concourse.bacc as bacc
import concourse.mybir as mybir
from concourse import tile

BF = ml_dtypes.bfloat16
F32, BF16 = mybir.dt.float32, mybir.dt.bfloat16
AF = mybir.ActivationFunctionType
OP = mybir.AluOpType

B, S, H = 2, 2048, 2048
NH, NKV, HD = 32, 8, 64
GR = NH // NKV
FF = 5632
EPS = 1e-5
N_CORES = 8
T = (B * S) // N_CORES        # 512 tokens per core
QT = T // 128                 # 4 query tiles per core
KB = S // 512                 # 4 key blocks of 512
SKT = S // 128                # 16 key tiles of 128
HPT = H // 128                # 16 hidden partition tiles
FFT = FF // 128               # 44 ff tiles
ROPE_BASE = 10000.0

_CACHE = {}


def _build_nc(shared, scales, analysis=False, no_collective=False):
    # analysis=True: single-core twin for offline TimelineSim (collective
    # replaced by equivalent local DMA traffic); no_collective=True: 8-core
    # build with the same local-DMA substitution (timing probe only — wrong
    # results). Neither is used for real runs.
    nc = bacc.Bacc("TRN2", target_bir_lowering=False, debug=False,
                   num_devices=(1 if analysis else N_CORES))
    din = {}
    def inp(name, shape, dt):
        din[name] = nc.dram_tensor(name, shape, dt, kind="ExternalInput").ap()
        return din[name]

    x_t   = inp("x_t",   [H, T], F32)          # x^T feature-major
    limits = inp("limits", [128, QT], F32)     # visible-key count per query row
    cos2  = inp("cos2",  [128, T], F32)        # cos stacked x2 (64-row pattern)
    srot  = inp("srot",  [128, T], F32)        # signed sin for rotate-half

    def cw(name):
        return nc.inline_tensor(np.ascontiguousarray(shared[name]), name=name).ap()

    # weights tile-major: row (mt*KT + kt)*128 + p, col c = w^T[kt*128+p, mt*128+c]
    qw_hi = cw("qw_hi")
    qw_lo = cw("qw_lo")
    kw_hi = cw("kw_hi")
    kw_lo = cw("kw_lo")
    vw    = cw("vw")
    ow    = cw("ow")
    gw    = cw("gw")
    uw    = cw("uw")
    dw    = cw("dw")
    out_d = nc.dram_tensor("out", [H, T], F32, kind="ExternalOutput").ap()

    with tile.TileContext(nc) as tc:
        with tc.tile_pool(name="const", bufs=1) as cpool, \
             tc.tile_pool(name="bb", bufs=1) as bpool, \
             tc.tile_pool(name="attn", bufs=1) as apool, \
             tc.tile_pool(name="kv", bufs=2) as kvpool, \
             tc.tile_pool(name="work", bufs=2) as wpool, \
             tc.tile_pool(name="pt", bufs=1) as ptpool, \
             tc.tile_pool(name="wt", bufs=2) as wtpool, \
             tc.tile_pool(name="small", bufs=4) as spool, \
             tc.tile_pool(name="psum", bufs=2, space="PSUM") as pspool, \
             tc.tile_pool(name="dram", bufs=1, space="DRAM") as dpool:

            ones128 = cpool.tile([128, 1], F32, tag="ones128")
            nc.vector.memset(ones128[:], 1.0)
            ones1 = cpool.tile([1, 128], F32, tag="ones1")
            nc.vector.memset(ones1[:], 1.0)
            cos_t = cpool.tile([128, T], F32, tag="cos2")
            nc.sync.dma_start(cos_t[:], cos2[:])
            srot_t = cpool.tile([128, T], F32, tag="srot")
            nc.sync.dma_start(srot_t[:], srot[:])

            eps_t = cpool.tile([1, 1], F32, tag="eps")
            nc.vector.memset(eps_t[:], EPS)

            # ---------- rmsnorm: stats from a DRAM fp32 [H, T] tensor ----------
            def rmsnorm_bcast(src_dram):
                ssum = pspool.tile([1, T], F32, tag="ps")
                for pt in range(HPT):
                    xt = wpool.tile([128, T], F32, tag="xin")
                    nc.sync.dma_start(xt[:], src_dram[pt * 128:(pt + 1) * 128, :])
                    sq = wpool.tile([128, T], F32, tag="hf")
                    nc.vector.tensor_tensor(sq[:], xt[:], xt[:], OP.mult)
                    nc.tensor.matmul(ssum[:], ones128[:], sq[:],
                                     start=(pt == 0), stop=(pt == HPT - 1))
                std = spool.tile([1, T], F32, tag="std", bufs=1)
                nc.scalar.activation(std[:], ssum[:], AF.Sqrt, bias=eps_t[:], scale=1.0 / H)
                rstd = spool.tile([1, T], F32, tag="rstd", bufs=1)
                nc.vector.reciprocal(rstd[:], std[:])
                bc = pspool.tile([128, T], F32, tag="ps")
                nc.tensor.matmul(bc[:], ones1[:], rstd[:], start=True, stop=True)
                bcs = wpool.tile([128, T], F32, tag="bcs", bufs=1)
                nc.vector.tensor_copy(bcs[:], bc[:])
                return bcs

            # ---------- phase 1: rmsnorm1 -> h hi/lo (bb slots 0..31) ----------
            bb = [bpool.tile([128, T], BF16, tag=f"bb{i}", name=f"bb{i}") for i in range(60)]
            h_hi = bb[0:HPT]
            h_lo = bb[HPT:2 * HPT]
            bc1 = rmsnorm_bcast(x_t)
            for pt in range(HPT):
                xt = wpool.tile([128, T], F32, tag="xin")
                nc.sync.dma_start(xt[:], x_t[pt * 128:(pt + 1) * 128, :])
                hf = wpool.tile([128, T], F32, tag="hf")
                nc.vector.tensor_tensor(hf[:], xt[:], bc1[:], OP.mult)
                nc.vector.tensor_copy(h_hi[pt][:], hf[:])
                nc.vector.scalar_tensor_tensor(h_lo[pt][:], hf[:], 1.0, h_hi[pt][:],
                                               OP.mult, OP.subtract)

            # ---------- helper: 3-term projection into psum [128, T] ----------
            def proj3(ps, w_hi_d, w_lo_d, mt):
                n_mm = 3 * HPT
                i = 0
                for cc in range(2):
                    wh = wtpool.tile([128, 128 * 8], BF16, tag="wh", name="wh")
                    wl = wtpool.tile([128, 128 * 8], BF16, tag="wl", name="wl")
                    r0 = (mt * 2 + cc) * 128
                    nc.sync.dma_start(wh[:], w_hi_d[r0:r0 + 128, :])
                    nc.sync.dma_start(wl[:], w_lo_d[r0:r0 + 128, :])
                    for j in range(8):
                        kt = cc * 8 + j
                        for wtile, htile in ((wh, h_hi[kt]), (wh, h_lo[kt]), (wl, h_hi[kt])):
                            nc.tensor.matmul(ps[:], wtile[:, j * 128:(j + 1) * 128],
                                             htile[:], start=(i == 0),
                                             stop=(i == n_mm - 1))
                            i += 1

            # ---------- helper: rope on psum [128, T] (2 heads) ----------
            def rope(ps):
                t1 = wpool.tile([128, T], F32, tag="rope1")
                nc.vector.tensor_tensor(t1[:], ps[:], cos_t[:], OP.mult)
                t2 = wpool.tile([128, T], F32, tag="rope2", bufs=1)
                for g in range(2):
                    o = g * 64
                    nc.vector.tensor_tensor(t2[o:o + 32, :], ps[o + 32:o + 64, :],
                                            srot_t[o:o + 32, :], OP.mult)
                    nc.vector.tensor_tensor(t2[o + 32:o + 64, :], ps[o:o + 32, :],
                                            srot_t[o + 32:o + 64, :], OP.mult)
                nc.vector.tensor_tensor(t1[:], t1[:], t2[:], OP.add)
                return t1

            # ---------- phase 2b: k proj + rope + split (own tokens) ----------
            k_hi_own, k_lo_own = [], []
            for mt in range(NKV * HD // 128):   # 4 tiles
                ps = pspool.tile([128, T], F32, tag="ps")
                proj3(ps, kw_hi, kw_lo, mt)
                kr = rope(ps)
                khi = wpool.tile([128, T], BF16, tag=f"khi{mt}", bufs=1)
                nc.vector.tensor_copy(khi[:], kr[:])
                klo = wpool.tile([128, T], BF16, tag=f"klo{mt}", bufs=1)
                nc.vector.scalar_tensor_tensor(klo[:], kr[:], 1.0, khi[:],
                                               OP.mult, OP.subtract)
                k_hi_own.append(khi)
                k_lo_own.append(klo)

            # ---------- phase 2c: v projection (token-major, bf16) ----------
            v_own = []
            for tmt in range(QT):   # 4 token tiles
                ps = pspool.tile([128, NKV * HD], F32, tag="ps")
                for kt in range(HPT):
                    wv = wtpool.tile([128, NKV * HD], BF16, tag="wv")
                    nc.sync.dma_start(wv[:], vw[kt * 128:(kt + 1) * 128, :])
                    nc.tensor.matmul(ps[:], h_hi[kt][:, tmt * 128:(tmt + 1) * 128],
                                     wv[:], start=(kt == 0), stop=(kt == HPT - 1))
                vt = wpool.tile([128, NKV * HD], BF16, tag=f"vown{tmt}", bufs=1)
                nc.vector.tensor_copy(vt[:], ps[:])
                v_own.append(vt)

            # ---------- phase 3: AllGather k_hi/k_lo/v ----------
            RPR = 1536  # bf16 rows per rank: khi 512, klo 512, v 512
            bounce_in = dpool.tile([RPR, 256], F32, tag="agin")
            bounce_out = dpool.tile([4 * RPR, 256], F32, tag="agout")
            bi_bf = bounce_in.bitcast(BF16)    # [1536, 512] bf16 view
            for mt in range(4):
                nc.sync.dma_start(bi_bf[mt * 128:(mt + 1) * 128, :], k_hi_own[mt][:])
                nc.sync.dma_start(bi_bf[512 + mt * 128:512 + (mt + 1) * 128, :],
                                  k_lo_own[mt][:])
                nc.sync.dma_start(bi_bf[1024 + mt * 128:1024 + (mt + 1) * 128, :],
                                  v_own[mt][:])
            if analysis or no_collective:
                for r in range(4):
                    nc.sync.dma_start(bounce_out[r * RPR:(r + 1) * RPR, :],
                                      bounce_in[:])
            else:
                nc.gpsimd.collective_compute(
                    "AllGather", OP.bypass,
                    replica_groups=[[0, 1, 2, 3], [4, 5, 6, 7]],
                    ins=[bounce_in.opt()],
                    outs=[bounce_out.opt()],
                )
            bo_bf = bounce_out.bitcast(BF16)   # [6144, 512] bf16 view

            # ---------- phase 2a: q proj + rope -> q_stack in DRAM ----------
            # Emitted AFTER the AllGather launch: q-proj depends only on h and
            # the q weights, so its ~0.4 ms of PE work overlaps the collective
            # flight instead of idling at the barrier.
            q_dram = dpool.tile([NH * 128, T], BF16, tag="qstack")
            for mt in range(HPT):        # 2 heads per mt
                ps = pspool.tile([128, T], F32, tag="ps")
                proj3(ps, qw_hi, qw_lo, mt)
                qr = rope(ps)
                qhi = wpool.tile([128, T], BF16, tag="qhi")
                nc.vector.tensor_copy(qhi[:], qr[:])
                qlo = wpool.tile([128, T], BF16, tag="qlo")
                nc.vector.scalar_tensor_tensor(qlo[:], qr[:], 1.0, qhi[:],
                                               OP.mult, OP.subtract)
                for g in range(2):
                    o = g * 64
                    hd_ = 2 * mt + g
                    nc.sync.dma_start(q_dram[hd_ * 128:hd_ * 128 + 64, :],
                                      qhi[o:o + 64, :])
                    nc.sync.dma_start(q_dram[hd_ * 128 + 64:(hd_ + 1) * 128, :],
                                      qlo[o:o + 64, :])

            # additive causal mask built on device: (col >= limit[row]) * -1e9
            limits_t = cpool.tile([128, QT], F32, tag="limits")
            nc.sync.dma_start(limits_t[:], limits[:])
            iota_f = apool.tile([128, S], F32, tag="iota")
            nc.gpsimd.iota(iota_f[:], [[1, S]], channel_multiplier=0,
                           allow_small_or_imprecise_dtypes=True)
            mask_sb = []
            for qt in range(QT):
                mk = apool.tile([128, S], BF16, tag=f"mask{qt}")
                nc.vector.tensor_scalar(mk[:], iota_f[:],
                                        limits_t[:, qt:qt + 1], -1e9,
                                        OP.is_ge, OP.mult)
                mask_sb.append(mk)

            attn = []    # 16 tiles [128, T] bf16: attn^T rows = head dims
            for mt in range(HPT):
                attn.append(apool.tile([128, T], BF16, tag=f"attn{mt}", name=f"attn{mt}"))

            # ---------- phase 5: attention ----------
            # Tokens are strided: core (b, lane l) owns global 128-blocks
            # {4j+l}; q tile j therefore sees only keys < (j+1)*512, the same
            # bound on EVERY core, so score/softmax/pv work is statically
            # skipped beyond it. Gathered k/v (rank-major) is re-ordered into
            # global key order at SBUF-load time: global block g lives in rank
            # g%4 at within-rank block g//4.
            for hd_ in range(NH):
                kvh = hd_ // GR
                if hd_ % GR == 0:
                    # stream this kv-head's k into SBUF in global key order
                    kd = kvpool.tile([128, S], BF16, tag="kdup", bufs=1)
                    kl = kvpool.tile([64, S], BF16, tag="klo", bufs=1)
                    for g in range(SKT):
                        r, a = g % 4, g // 4
                        cs = slice(a * 128, (a + 1) * 128)
                        src_hi = bo_bf[r * RPR + kvh * 64: r * RPR + kvh * 64 + 64, cs]
                        src_lo = bo_bf[r * RPR + 512 + kvh * 64:
                                       r * RPR + 512 + kvh * 64 + 64, cs]
                        ds = slice(g * 128, (g + 1) * 128)
                        nc.sync.dma_start(kd[0:64, ds], src_hi)
                        nc.sync.dma_start(kd[64:128, ds], src_hi)
                        nc.sync.dma_start(kl[:, ds], src_lo)
                    v_kv = []
                    for g in range(SKT):
                        r, a = g % 4, g // 4
                        vt = kvpool.tile([128, HD], BF16, tag=f"vk{g}", name=f"vk{g}")
                        nc.sync.dma_start(
                            vt[:],
                            bo_bf[r * RPR + 1024 + a * 128: r * RPR + 1024 + (a + 1) * 128,
                                  kvh * 64:(kvh + 1) * 64])
                        v_kv.append(vt)
                pt_all = ptpool.tile([128, SKT * T], BF16, tag="ptall", name="ptall")
                pt3 = pt_all[:].rearrange("p (a c) -> p a c", a=SKT, c=T)
                qs = wpool.tile([128, T], BF16, tag="qslice")
                nc.sync.dma_start(qs[:], q_dram[hd_ * 128:(hd_ + 1) * 128, :])
                for qt in range(QT):
                    w = (qt + 1) * 512          # visible key width for this q tile
                    nkt = 4 * (qt + 1)
                    qc = slice(qt * 128, (qt + 1) * 128)
                    ps = pspool.tile([128, S], F32, tag="ps")
                    for kb in range(qt + 1):
                        sl = slice(kb * 512, (kb + 1) * 512)
                        nc.tensor.matmul(ps[:, sl], qs[:, qc], kd[:, sl],
                                         start=True, stop=False)
                        nc.tensor.matmul(ps[:, sl], qs[0:64, qc], kl[:, sl],
                                         start=False, stop=True)
                    # in-place mask add on PSUM, then row max, exp, normalize
                    nc.vector.scalar_tensor_tensor(ps[:, 0:w], ps[:, 0:w], 1.0,
                                                   mask_sb[qt][:, 0:w],
                                                   OP.mult, OP.add)
                    mx = spool.tile([128, 1], F32, tag="mx")
                    nc.vector.tensor_reduce(mx[:], ps[:, 0:w],
                                            axis=mybir.AxisListType.X, op=OP.max)
                    nmx = spool.tile([128, 1], F32, tag="nmx")
                    nc.vector.tensor_scalar_mul(nmx[:], mx[:], -1.0)
                    pbf = wpool.tile([128, S], BF16, tag="pbf")
                    sume = spool.tile([128, 1], F32, tag="sume")
                    nc.scalar.activation(pbf[:, 0:w], ps[:, 0:w], AF.Exp, bias=nmx[:],
                                         scale=1.0, accum_out=sume[:])
                    rsum = spool.tile([128, 1], F32, tag="rsum")
                    nc.vector.reciprocal(rsum[:], sume[:])
                    nc.vector.tensor_scalar_mul(pbf[:, 0:w], pbf[:, 0:w], rsum[:])
                    nc.sync.dma_start_transpose(
                        pt3[:, 0:nkt, qt * 128:(qt + 1) * 128],
                        pbf[:, 0:w])
                pav = pspool.tile([64, T], F32, tag="ps")
                for qt in range(QT):
                    nkt = 4 * (qt + 1)
                    qcol = slice(qt * 128, (qt + 1) * 128)
                    for kt in range(nkt):
                        nc.tensor.matmul(
                            pav[:, qcol], v_kv[kt][:],
                            pt_all[:, kt * T + qt * 128:kt * T + (qt + 1) * 128],
                            start=(kt == 0), stop=(kt == nkt - 1))
                o = (hd_ % 2) * 64
                nc.vector.tensor_copy(attn[hd_ // 2][o:o + 64, :], pav[:])

            # ---------- phase 6: o-proj + residual -> hid in DRAM ----------
            hid_d = dpool.tile([H, T], F32, tag="hid")
            for mt in range(HPT):
                ps = pspool.tile([128, T], F32, tag="ps")
                for cc in range(2):
                    wo = wtpool.tile([128, 128 * 8], BF16, tag="wh", name="wo")
                    r0 = (mt * 2 + cc) * 128
                    nc.sync.dma_start(wo[:], ow[r0:r0 + 128, :])
                    for j in range(8):
                        kt = cc * 8 + j
                        nc.tensor.matmul(ps[:], wo[:, j * 128:(j + 1) * 128],
                                         attn[kt][:], start=(kt == 0),
                                         stop=(kt == HPT - 1))
                xt = wpool.tile([128, T], F32, tag="xin")
                nc.sync.dma_start(xt[:], x_t[mt * 128:(mt + 1) * 128, :])
                ht = wpool.tile([128, T], F32, tag="hf")
                nc.vector.tensor_tensor(ht[:], ps[:], xt[:], OP.add)
                nc.sync.dma_start(hid_d[mt * 128:(mt + 1) * 128, :], ht[:])

            # ---------- phase 7: rmsnorm2 -> h2 (bb slots 0..15) ----------
            h2 = bb[0:HPT]
            bc2 = rmsnorm_bcast(hid_d)
            for pt in range(HPT):
                xt = wpool.tile([128, T], F32, tag="xin")
                nc.sync.dma_start(xt[:], hid_d[pt * 128:(pt + 1) * 128, :])
                hf = wpool.tile([128, T], F32, tag="hf")
                nc.vector.tensor_tensor(hf[:], xt[:], bc2[:], OP.mult)
                nc.vector.tensor_copy(h2[pt][:], hf[:])

            # ---------- phase 8: gate/up + silu -> act (bb slots 16..59) ----------
            act = bb[HPT:HPT + FFT]
            for ft in range(FFT):
                psg = pspool.tile([128, T], F32, tag="ps")
                psu = pspool.tile([128, T], F32, tag="ps")
                for cc in range(2):
                    wg = wtpool.tile([128, 128 * 8], BF16, tag="wh", name="wg")
                    wu = wtpool.tile([128, 128 * 8], BF16, tag="wl", name="wu")
                    r0 = (ft * 2 + cc) * 128
                    nc.sync.dma_start(wg[:], gw[r0:r0 + 128, :])
                    nc.sync.dma_start(wu[:], uw[r0:r0 + 128, :])
                    for j in range(8):
                        kt = cc * 8 + j
                        nc.tensor.matmul(psg[:], wg[:, j * 128:(j + 1) * 128],
                                         h2[kt][:], start=(kt == 0), stop=(kt == HPT - 1))
                        nc.tensor.matmul(psu[:], wu[:, j * 128:(j + 1) * 128],
                                         h2[kt][:], start=(kt == 0), stop=(kt == HPT - 1))
                gs = wpool.tile([128, T], BF16, tag="gs")
                nc.scalar.activation(gs[:], psg[:], AF.Silu)
                nc.vector.tensor_tensor(act[ft][:], gs[:], psu[:], OP.mult)

            # ---------- phase 9: down + residual -> out ----------
            for mt in range(HPT):
                ps = pspool.tile([128, T], F32, tag="ps")
                for kc in range(4):          # 11 kt per chunk
                    wd = wtpool.tile([128, 128 * 11], BF16, tag="wd")
                    r0 = (mt * 4 + kc) * 128
                    nc.sync.dma_start(wd[:], dw[r0:r0 + 128, :])
                    for j in range(11):
                        kt = kc * 11 + j
                        nc.tensor.matmul(ps[:], wd[:, j * 128:(j + 1) * 128],
                                         act[kt][:], start=(kt == 0),
                                         stop=(kt == FFT - 1))
                xt = wpool.tile([128, T], F32, tag="xin")
                nc.sync.dma_start(xt[:], hid_d[mt * 128:(mt + 1) * 128, :])
                ot = wpool.tile([128, T], F32, tag="hf")
                nc.vector.tensor_tensor(ot[:], ps[:], xt[:], OP.add)
                nc.sync.dma_start(out_d[mt * 128:(mt + 1) * 128, :], ot[:])

    nc.compile()
    return nc


def _shared_weights(inputs):
    kk = np.float32(inputs["kk"])
    aa = np.float32(inputs["aa"])
    def binw(w):
        return (aa * np.clip(kk * np.asarray(w, dtype=np.float32), -1.0, 1.0))
    ln1 = np.asarray(inputs["ln1_w"], dtype=np.float32)
    ln2 = np.asarray(inputs["ln2_w"], dtype=np.float32)
    qw = binw(inputs["q_w"]) * ln1[None, :] / np.float32(math.sqrt(HD))
    kw = binw(inputs["k_w"]) * ln1[None, :]
    vw = binw(inputs["v_w"]) * ln1[None, :]
    ow = binw(inputs["o_w"])
    gw = binw(inputs["gate_w"]) * ln2[None, :]
    uw = binw(inputs["up_w"]) * ln2[None, :]
    dw = binw(inputs["down_w"])

    def split(w):
        hi = w.astype(BF)
        lo = (w - hi.astype(np.float32)).astype(BF)
        return np.ascontiguousarray(hi), np.ascontiguousarray(lo)

    def panels(wt, kpp):
        # wt: [K, M] -> panel-major [n_mt*n_pan*128, kpp*128] where panel
        # (mt, pan) row p, col j*128+c = wt[(pan*kpp+j)*128+p, mt*128+c].
        # Each SBUF weight tile then fills with ONE dma whose per-partition
        # data is contiguous (kpp*128 elems), minimizing DMA descriptors.
        K, M = wt.shape
        n_kt, n_mt = K // 128, M // 128
        n_pan = n_kt // kpp
        w5 = wt.reshape(n_pan, kpp, 128, n_mt, 128).transpose(3, 0, 2, 1, 4)
        return np.ascontiguousarray(w5.reshape(n_mt * n_pan * 128, kpp * 128))

    qw_hi, qw_lo = split(qw.T)     # [H, H]
    kw_hi, kw_lo = split(kw.T)     # [H, 512]
    shared = {
        "qw_hi": panels(qw_hi, 8), "qw_lo": panels(qw_lo, 8),
        "kw_hi": panels(kw_hi, 8), "kw_lo": panels(kw_lo, 8),
        "vw": np.ascontiguousarray(vw.T.astype(BF)),
        "ow": panels(ow.T.astype(BF), 8),
        "gw": panels(gw.T.astype(BF), 8),
        "uw": panels(uw.T.astype(BF), 8),
        "dw": panels(dw.T.astype(BF), 11),
    }
    return shared, {}


def _token_idx(c):
    """Global token indices (within batch b=c//4) owned by core c, in on-core
    order: strided 128-blocks {4j + (c%4)} so every core has the same causal
    profile per q tile."""
    l = c % 4
    return np.concatenate([np.arange((4 * j + l) * 128, (4 * j + l + 1) * 128)
                           for j in range(QT)])


def _activation_maps(inputs):
    x = np.asarray(inputs["hidden_states"], dtype=np.float32)
    mask = np.asarray(inputs["attention_mask"], dtype=np.float32)
    pos = np.asarray(inputs["position_ids"], dtype=np.int32)

    in_maps = []
    for c in range(N_CORES):
        b = c // 4
        sl = _token_idx(c)
        inv = (1.0 / (ROPE_BASE ** (np.arange(0, HD, 2, dtype=np.float32) / np.float32(HD))))
        fr = pos[b, sl].astype(np.float32)[:, None] * inv[None, :]   # [T, 32]
        emb = np.concatenate([fr, fr], axis=-1)                      # [T, 64]
        cos = np.cos(emb).astype(np.float32).T                       # [64, T]
        sin = np.sin(emb).astype(np.float32).T                       # [64, T]
        srot = np.concatenate([-sin[0:32], sin[32:64]], axis=0)      # [64, T]
        # visible-key count per query row, from the actual input mask
        vis = (mask[b, 0, sl, :] > -0.5).sum(axis=-1).astype(np.float32)  # [T]
        limits = np.ascontiguousarray(vis.reshape(QT, 128).T)             # [128, QT]
        in_maps.append({
            "x_t": np.ascontiguousarray(x[b, sl].T),
            "limits": limits,
            "cos2": np.ascontiguousarray(np.concatenate([cos, cos], axis=0)),
            "srot": np.ascontiguousarray(np.concatenate([srot, srot], axis=0)),
        })
    return in_maps


def _weights_key(shared, scales):
    h = hashlib.blake2b(digest_size=16)
    for k in sorted(shared):
        h.update(k.encode())
        h.update(np.ascontiguousarray(shared[k]).tobytes())
    for k in sorted(scales):
        h.update(k.encode())
        h.update(np.float32(scales[k]).tobytes())
    return h.hexdigest()


def prepare(inputs):
    """Build (or reuse) the compiled nc + jitted runner for these weights.

    Returns (nc, in_maps) where in_maps carry only the per-core activations.
    """
    shared, scales = _shared_weights(inputs)
    key = _weights_key(shared, scales)
    if _CACHE.get("key") != key:
        _CACHE.clear()
        _CACHE["key"] = key
        _CACHE["nc"] = _build_nc(shared, scales)
    return _CACHE["nc"], _activation_maps(inputs)


def _get_runner(nc):
    """Jitted SPMD runner (cached; safe for repeat kernel() calls even though
    the neuron lowering mutates Const allocations in place)."""
    if "runner" in _CACHE:
        return _CACHE["runner"]
    import jax
    from jax.sharding import Mesh, PartitionSpec
    from jax.experimental.shard_map import shard_map
    from concourse.bass2jax import (_bass_exec_p, install_neuronx_cc_hook,
                                    partition_id_tensor)
    install_neuronx_cc_hook()

    partition_name = nc.partition_id_tensor.name if nc.partition_id_tensor else None
    in_names, out_names, out_avals = [], [], []
    for alloc in nc.m.functions[0].allocations:
        if not isinstance(alloc, mybir.MemoryLocationSet):
            continue
        name = alloc.memorylocations[0].name
        if alloc.kind == "ExternalInput":
            if name != partition_name:
                in_names.append(name)
        elif alloc.kind == "ExternalOutput":
            out_names.append(name)
            shape = tuple(alloc.tensor_shape)
            dtype = mybir.dt.np(alloc.dtype)
            out_avals.append(jax.core.ShapedArray(shape, dtype))
    # outputs get fresh NKI hbm buffers in the neuron lowering; no zero-init
    # operands needed (the kernel writes every element of out)
    all_in_names = list(in_names)
    if partition_name is not None:
        all_in_names.append(partition_name)

    def _body(*args):
        operands = list(args)
        if partition_name is not None:
            operands.append(partition_id_tensor())
        outs = _bass_exec_p.bind(
            *operands,
            out_avals=tuple(out_avals),
            in_names=tuple(all_in_names),
            out_names=tuple(out_names),
            lowering_input_output_aliases=(),
            sim_require_finite=True,
            sim_require_nnan=True,
            nc=nc,
        )
        return tuple(outs)

    devices = jax.devices()[:N_CORES]
    mesh = Mesh(np.asarray(devices), ("core",))
    fn = jax.jit(shard_map(_body, mesh=mesh,
                           in_specs=(PartitionSpec("core"),) * len(in_names),
                           out_specs=(PartitionSpec("core"),) * len(out_names),
                           check_rep=False), keep_unused=True)
    runner = {"fn": fn, "in_names": in_names, "out_names": out_names,
              "jax": jax}
    _CACHE["runner"] = runner
    return runner


def kernel(**inputs):
    nc, in_maps = prepare(inputs)
    r = _get_runner(nc)
    jax = r["jax"]
    concat_in = [np.concatenate([np.asarray(in_maps[c][k]) for c in range(N_CORES)],
                                axis=0) for k in r["in_names"]]
    outs = jax.block_until_ready(r["fn"](*concat_in))
    oidx = r["out_names"].index("out")
    res = np.asarray(outs[oidx]).reshape(N_CORES, H, T)
    out = np.empty((B, S, H), dtype=np.float32)
    for c in range(N_CORES):
        out[c // 4, _token_idx(c), :] = res[c].T
    return out
